# revision 4
# baseline (speedup 1.0000x reference)
"""Trainium2 Bass kernel for nn_Encoder (NRI-style GNN message-passing encoder).

Math (see reference):
  h  = MLP1(x)                       [B,N,H]   N=64 nodes, H=128
  e  = MLP2(node2edge(h))            [B,E,H]   E=4032 edges (fully connected, no self)
  n  = MLP3(edge2node(e))            [B,N,H]
  e2 = MLP4([node2edge(n), e])       [B,E,H]
  out= e2 @ wout + bout              [B,E,16]

Distribution: data-parallel over batch, 8 items per core x 8 cores.

Kernel-side restructuring (validated in numpy, proto.py):
- Activations kept feature-major [features(partition), tokens(free)]; MLP weights
  [fan_in, fan_out] are used directly as the PE stationary operand (lhsT).
- Edges reordered as e' = (d-1)*64 + i  <=>  (sender=i, receiver=(i+d)%64),
  d=1..63. With hT9 = tile(hT_b, 9) [128, 576]:
    senders  chunk of 8 d-blocks: hT9[:, 0:512]               (plain AP)
    receivers chunk:              offset d0, AP [[65,8],[1,64]]
  so node2edge costs nothing - it is folded into the matmul access patterns.
- edge2node folded through w2b: agg = (S @ w2b + 63*b2b)/63.000001 where
  S[j] = sum_d h2block_d[(j-d)%64], computed as 63 PSUM-accumulated
  exact-identity bf16 matmuls over a doubled bf16 copy of h2.
- Final linear folded: w4out = w4b@wout, b4out = b4b@wout + bout.
- float32r (reduced-precision fast PE mode, full fp32 storage) for the wide
  matmuls; all f32r operands are produced by compute instructions (rounding).

The harness calls kernel(**inputs) with the full unsharded inputs.
"""
import sys

sys.path.insert(0, "/opt/trn_rl_repo")

import numpy as np

import concourse.bass as bass
from concourse import bacc
import concourse.mybir as mybir
import concourse.tile as tile
from concourse.bass_utils import run_bass_kernel_spmd

F32 = mybir.dt.float32
F32R = mybir.dt.float32r
BF16 = mybir.dt.bfloat16

N_NODES = 64
N_EDGES = 4032
BATCH = 64
N_IN = 64
H = 128
N_OUT = 16
N_CORES = 8
B_LOC = BATCH // N_CORES  # 8 items per core

# chunking of the 63 d-blocks into moving-operand chunks
CHUNKS = [(1 + 8 * c, 8 if c < 7 else 7) for c in range(8)]  # (d0, nblocks)

_AF = mybir.ActivationFunctionType


def _edge_perm():
    """perm[p] = original edge index of e'-ordered edge p."""
    d, i = np.meshgrid(np.arange(1, 64), np.arange(64), indexing="ij")
    j = (i + d) % 64
    return (i * 63 + (j - (j > i))).reshape(-1)


def _ap(t, off, dims):
    return bass.AP(tensor=t.tensor, offset=t.offset + off, ap=[t.ap[0]] + dims)


def build_kernel():
    nc = bacc.Bacc("TRN2", target_bir_lowering=False, debug=False)

    def din(name, shape):
        return nc.dram_tensor(name, shape, F32, kind="ExternalInput").ap()

    xT_d = din("xT", [N_IN, B_LOC * N_NODES])  # feature-major x shard
    w1a_d = din("w1a", [N_IN, H])
    b1a_d = din("b1a", [H, 1])
    w1b_d = din("w1b", [H, H])
    b1b_d = din("b1b", [H, 1])
    w2s_d = din("w2s", [H, H])
    w2r_d = din("w2r", [H, H])
    b2a_d = din("b2a", [H, 1])
    w2b_d = din("w2b", [H, H])
    b2b_d = din("b2b", [H, 1])
    b2n_d = din("b2n", [H, 1])  # folded edge2node bias 63*b2b/63.000001
    w3a_d = din("w3a", [H, H])
    b3a_d = din("b3a", [H, 1])
    w3b_d = din("w3b", [H, H])
    b3b_d = din("b3b", [H, 1])
    w4s_d = din("w4s", [H, H])
    w4r_d = din("w4r", [H, H])
    w4k_d = din("w4k", [H, H])
    b4a_d = din("b4a", [H, 1])
    w4o_d = din("w4o", [H, N_OUT])  # w4b @ wout
    b4o_d = din("b4o", [N_OUT, 1])  # b4b @ wout + bout
    id_d = din("ident", [H, H])

    y_d = nc.dram_tensor("y", [B_LOC, N_OUT, N_EDGES], F32, kind="ExternalOutput").ap()

    scale2n = 1.0 / (63.0 + 1e-6)

    with tile.TileContext(nc) as tc:
        with (
            tc.tile_pool(name="wp", bufs=1) as wp,  # weights, resident
            tc.tile_pool(name="hp", bufs=3) as hp,  # per-item small tiles
            tc.tile_pool(name="big", bufs=2) as big,  # per-item wide tensors
            tc.tile_pool(name="pbig", bufs=2, space="PSUM") as pbig,  # [128,512] mm
            tc.tile_pool(name="pout", bufs=2, space="PSUM") as pout,  # 2nd-layer mm
            tc.tile_pool(name="psml", bufs=2, space="PSUM") as psml,  # small mm
            tc.tile_pool(name="op", bufs=2) as op,
        ):
            # ---- resident weights: DMA fp32, round to f32r via DVE copy ----
            def wload(d, shape):
                raw = wp.tile(shape, F32, tag=d.tensor.name + "_raw")
                nc.sync.dma_start(raw, d)
                t = wp.tile(shape, F32R, tag=d.tensor.name + "_sb")
                nc.vector.tensor_copy(t, raw)
                return t

            w1a = wload(w1a_d, [N_IN, H])
            w1b = wload(w1b_d, [H, H])
            w2s = wload(w2s_d, [H, H])
            w2r = wload(w2r_d, [H, H])
            w2b = wload(w2b_d, [H, H])
            w3a = wload(w3a_d, [H, H])
            w3b = wload(w3b_d, [H, H])
            w4s = wload(w4s_d, [H, H])
            w4r = wload(w4r_d, [H, H])
            w4k = wload(w4k_d, [H, H])
            w4o = wload(w4o_d, [H, N_OUT])
            biases = {}
            for d in (b1a_d, b1b_d, b2a_d, b2b_d, b2n_d, b3a_d, b3b_d, b4a_d):
                t = wp.tile([H, 1], F32, tag=d.tensor.name + "_sb")
                nc.sync.dma_start(t, d)
                biases[d.tensor.name] = t
            b4o = wp.tile([N_OUT, 1], F32)
            nc.sync.dma_start(b4o, b4o_d)
            id32 = wp.tile([H, H], F32)
            nc.sync.dma_start(id32, id_d)
            idbf = wp.tile([H, H], BF16)
            nc.vector.tensor_copy(idbf, id32)

            xTraw = wp.tile([N_IN, B_LOC * N_NODES], F32)
            nc.sync.dma_start(xTraw, xT_d)
            xT = wp.tile([N_IN, B_LOC * N_NODES], F32R)
            nc.vector.tensor_copy(xT, xTraw)

            # ---- MLP1 over all items at once (512 tokens) ----
            p1 = psml.tile([H, B_LOC * N_NODES], F32, tag="psml")
            nc.tensor.matmul(p1, w1a, xT, start=True, stop=True)
            h1T = hp.tile([H, B_LOC * N_NODES], F32R, tag="h1T")
            nc.scalar.activation(h1T, p1, _AF.Relu, bias=biases["b1a"])
            p2 = psml.tile([H, B_LOC * N_NODES], F32, tag="psml")
            nc.tensor.matmul(p2, w1b, h1T, start=True, stop=True)
            hT = hp.tile([H, B_LOC * N_NODES], F32, tag="hT")
            nc.scalar.activation(hT, p2, _AF.Identity, bias=biases["b1b"])

            # ---- per item pipeline ----
            for b in range(B_LOC):
                # hT9: tile item's hT 9x along free dim -> [128, 576] (rounds to f32r)
                hT9 = hp.tile([H, 9 * N_NODES], F32R, tag="hT9")
                nc.vector.tensor_copy(
                    hT9, _ap(hT, b * N_NODES, [[0, 9], [1, N_NODES]])
                )

                h2T = big.tile([H, N_EDGES], F32R, tag="h2T")
                h2d = big.tile([H, 63 * 2 * N_NODES], BF16, tag="h2d")  # doubled blocks
                e2T = big.tile([H, N_EDGES], F32R, tag="e2T")  # x_skip

                for d0, nb in CHUNKS:
                    ncols = nb * N_NODES
                    s0 = (d0 - 1) * N_NODES
                    pmm = pbig.tile([H, 512], F32, tag="pbig")
                    # senders + receivers contributions (node2edge folded in APs)
                    nc.tensor.matmul(
                        pmm[:, 0:ncols], w2s, hT9[:, 0:ncols], start=True, stop=False
                    )
                    nc.tensor.matmul(
                        pmm[:, 0:ncols], w2r,
                        _ap(hT9, d0, [[65, nb], [1, N_NODES]]),
                        start=False, stop=True,
                    )
                    # relu + bias: f32r copy + two bf16 doubled copies
                    nc.scalar.activation(
                        h2T[:, s0 : s0 + ncols], pmm[:, 0:ncols], _AF.Relu,
                        bias=biases["b2a"],
                    )
                    nc.scalar.activation(
                        _ap(h2d, (d0 - 1) * 128, [[128, nb], [1, N_NODES]]),
                        pmm[:, 0:ncols], _AF.Relu, bias=biases["b2a"],
                    )
                    nc.scalar.activation(
                        _ap(h2d, (d0 - 1) * 128 + N_NODES, [[128, nb], [1, N_NODES]]),
                        pmm[:, 0:ncols], _AF.Relu, bias=biases["b2a"],
                    )
                    # second MLP2 layer -> e2T (x_skip)
                    pe2 = pout.tile([H, 512], F32, tag="pout")
                    nc.tensor.matmul(
                        pe2[:, 0:ncols], w2b, h2T[:, s0 : s0 + ncols],
                        start=True, stop=True,
                    )
                    nc.vector.tensor_scalar_add(
                        e2T[:, s0 : s0 + ncols], pe2[:, 0:ncols], biases["b2b"]
                    )

                # ---- edge2node: 63 identity matmuls over shifted doubled blocks ----
                pS = psml.tile([H, N_NODES], F32, tag="psml")
                for d in range(1, 64):
                    nc.tensor.matmul(
                        pS,
                        idbf,
                        _ap(h2d, (d - 1) * 128 + (64 - d), [[1, N_NODES]]),
                        start=(d == 1),
                        stop=(d == 63),
                    )
                S = hp.tile([H, N_NODES], F32R, tag="S")
                nc.vector.tensor_copy(S, pS)
                pagg = psml.tile([H, N_NODES], F32, tag="psml")
                nc.tensor.matmul(pagg, w2b, S, start=True, stop=True)
                aggT = hp.tile([H, N_NODES], F32R, tag="aggT")
                nc.scalar.activation(
                    aggT, pagg, _AF.Identity, bias=biases["b2n"], scale=scale2n
                )

                # ---- MLP3 ----
                pn1 = psml.tile([H, N_NODES], F32, tag="psml")
                nc.tensor.matmul(pn1, w3a, aggT, start=True, stop=True)
                n1T = hp.tile([H, N_NODES], F32R, tag="n1T")
                nc.scalar.activation(n1T, pn1, _AF.Relu, bias=biases["b3a"])
                pn2 = psml.tile([H, N_NODES], F32, tag="psml")
                nc.tensor.matmul(pn2, w3b, n1T, start=True, stop=True)
                nT = hp.tile([H, N_NODES], F32, tag="nT")
                nc.scalar.activation(nT, pn2, _AF.Identity, bias=biases["b3b"])
                nT9 = hp.tile([H, 9 * N_NODES], F32R, tag="nT9")
                nc.vector.tensor_copy(nT9, _ap(nT, 0, [[0, 9], [1, N_NODES]]))

                # ---- MLP4 + folded output ----
                outT = op.tile([N_OUT, N_EDGES], F32, tag="outT")
                for d0, nb in CHUNKS:
                    ncols = nb * N_NODES
                    s0 = (d0 - 1) * N_NODES
                    pm4 = pbig.tile([H, 512], F32, tag="pbig")
                    nc.tensor.matmul(
                        pm4[:, 0:ncols], w4s, nT9[:, 0:ncols], start=True, stop=False
                    )
                    nc.tensor.matmul(
                        pm4[:, 0:ncols], w4r,
                        _ap(nT9, d0, [[65, nb], [1, N_NODES]]),
                        start=False, stop=False,
                    )
                    nc.tensor.matmul(
                        pm4[:, 0:ncols], w4k, e2T[:, s0 : s0 + ncols],
                        start=False, stop=True,
                    )
                    h4T = hp.tile([H, 512], F32R, tag="h4T")
                    nc.scalar.activation(
                        h4T[:, 0:ncols], pm4[:, 0:ncols], _AF.Relu, bias=biases["b4a"]
                    )
                    po = pout.tile([N_OUT, 512], F32, tag="pout")
                    nc.tensor.matmul(
                        po[:, 0:ncols], w4o, h4T[:, 0:ncols], start=True, stop=True
                    )
                    nc.vector.tensor_scalar_add(
                        outT[:, s0 : s0 + ncols], po[:, 0:ncols], b4o
                    )
                nc.sync.dma_start(y_d[b], outT)

    nc.compile()
    return nc


_CACHE = {}


def _get_nc():
    if "nc" not in _CACHE:
        _CACHE["nc"] = build_kernel()
        _CACHE["perm"] = _edge_perm()
    return _CACHE["nc"], _CACHE["perm"]


def make_in_maps(inputs):
    """Host-side precompute + shard: returns per-core input maps."""
    x = np.ascontiguousarray(inputs["x"], np.float32)  # [64, 64, 64]
    w4out = inputs["w4b"] @ inputs["wout"]  # [128, 16]
    b4out = inputs["b4b"] @ inputs["wout"] + inputs["bout"]  # [16]
    common = {
        "w1a": inputs["w1a"],
        "b1a": inputs["b1a"][:, None],
        "w1b": inputs["w1b"],
        "b1b": inputs["b1b"][:, None],
        "w2s": inputs["w2a"][:H],
        "w2r": inputs["w2a"][H:],
        "b2a": inputs["b2a"][:, None],
        "w2b": inputs["w2b"],
        "b2b": inputs["b2b"][:, None],
        "b2n": (63.0 * inputs["b2b"] / (63.0 + 1e-6))[:, None],
        "w3a": inputs["w3a"],
        "b3a": inputs["b3a"][:, None],
        "w3b": inputs["w3b"],
        "b3b": inputs["b3b"][:, None],
        "w4s": inputs["w4a"][:H],
        "w4r": inputs["w4a"][H : 2 * H],
        "w4k": inputs["w4a"][2 * H :],
        "b4a": inputs["b4a"][:, None],
        "w4o": w4out,
        "b4o": b4out[:, None],
        "ident": np.eye(H, dtype=np.float32),
    }
    common = {k: np.ascontiguousarray(v, np.float32) for k, v in common.items()}

    in_maps = []
    for c in range(N_CORES):
        xs = x[c * B_LOC : (c + 1) * B_LOC]  # [8, 64, 64]
        xT = np.ascontiguousarray(xs.reshape(B_LOC * N_NODES, N_IN).T)
        in_maps.append({**common, "xT": xT})
    return in_maps


def gather_out(results, perm):
    """Merge per-core results (list of dicts with 'y') into full output."""
    out = np.empty((BATCH, N_EDGES, N_OUT), np.float32)
    inv = np.empty_like(perm)
    inv[perm] = np.arange(N_EDGES)
    for c in range(N_CORES):
        y = results[c]["y"]  # [B_LOC, N_OUT, N_EDGES] in e'-order
        out[c * B_LOC : (c + 1) * B_LOC] = np.asarray(y).transpose(0, 2, 1)[:, inv, :]
    return out


def kernel(**inputs):
    nc, perm = _get_nc()
    in_maps = make_in_maps(inputs)
    res = run_bass_kernel_spmd(nc, in_maps, core_ids=list(range(N_CORES)))
    return gather_out(res.results, perm)


# revision 9
# speedup vs baseline: 1.5955x; 1.5955x over previous
"""Trainium2 Bass kernel for nn_Encoder (NRI-style GNN message-passing encoder).

Reference math:
  h  = MLP1(x)                       [B,N,H]   N=64 nodes, H=128
  e  = MLP2(node2edge(h))            [B,E,H]   E=4032 edges (fully connected)
  n  = MLP3(edge2node(e))            [B,N,H]
  e2 = MLP4([node2edge(n), e])       [B,E,H]
  out= e2 @ wout + bout              [B,E,16]

Distribution: data-parallel over batch, 8 items per core x 8 cores.

Kernel restructuring (validated in numpy + CoreSim):
- Feature-major activations [features(partition), tokens(free)]; weights
  [fan_in, fan_out] used directly as PE stationary (lhsT).
- Edge reorder (receiver-major): e'' = (d-1)*64 + j  <=>  edge
  (sender=(j-d)%64, receiver=j), d=1..63. With hT9 = tile(hT_b, 9):
    receivers chunk: hT9[:, 0:ncols]                  (contiguous)
    senders  chunk: offset 64-d0, AP [[63,nb],[1,64]] (shifted windows)
  so node2edge is free, folded into matmul access patterns.
- edge2node = plain strided reduce on h2T: S[j] = sum_d h2T[:, (d-1)*64+j]
  (one DVE tensor_reduce per item, no materialization).
- x_skip eliminated: e appears only as MLP4's third K-block, so
  W2bk = w2b @ w4a_k and b4a' = b4a + b2b @ w4a_k are folded host-side and
  MLP4 reads h2T directly (exact algebra, fewer roundings).
- Final linear folded: w4o = w4b @ wout; its bias b4o is added on HOST.
- MLP4-out chunks packed 3-per-PSUM-bank at base partitions 0/32/64.
- float32r (fast PE mode) everywhere; f32r operands produced by rounding
  compute instructions as the compiler requires.

The harness calls kernel(**inputs) with full unsharded inputs.
"""
import sys

sys.path.insert(0, "/opt/trn_rl_repo")

import numpy as np

import concourse.bass as bass
from concourse import bacc
import concourse.mybir as mybir
import concourse.tile as tile
from concourse.bass_utils import run_bass_kernel_spmd

F32 = mybir.dt.float32
F32R = mybir.dt.float32r

N_NODES = 64
N_EDGES = 4032
BATCH = 64
N_IN = 64
H = 128
N_OUT = 16
N_CORES = 8
B_LOC = BATCH // N_CORES

# 63 s-blocks (s = 1..63; block s holds edges (sender=(j+s)%64, receiver=j))
# -> 8 uniform chunks of 8 blocks; the last chunk starts at s=56, overlapping
# chunk 6 by one block so every chunk is exactly 512 columns.
CHUNKS = [(1 + 8 * c, 8) for c in range(7)] + [(56, 8)]

_AF = mybir.ActivationFunctionType
_ALU = mybir.AluOpType


def _edge_perm():
    """perm[p] = original edge index of reordered edge p = (s-1)*64 + j,
    which is edge (sender=(j+s)%64, receiver=j)."""
    s, j = np.meshgrid(np.arange(1, 64), np.arange(64), indexing="ij")
    i = (j + s) % 64
    return (i * 63 + (j - (j > i))).reshape(-1)


def _ap(t, off, dims):
    return bass.AP(tensor=t.tensor, offset=t.offset + off, ap=[t.ap[0]] + dims)


def build_kernel():
    nc = bacc.Bacc("TRN2", target_bir_lowering=False, debug=False)

    def din(name, shape):
        return nc.dram_tensor(name, shape, F32, kind="ExternalInput").ap()

    xT_d = din("xT", [N_IN, B_LOC * N_NODES])
    w1a_d = din("w1a", [N_IN, H])
    b1a_d = din("b1a", [H, 1])
    w1b_d = din("w1b", [H, H])
    b1b_d = din("b1b", [H, 1])
    w2s_d = din("w2s", [H, H])
    w2r_d = din("w2r", [H, H])
    b2a_d = din("b2a", [H, 1])
    w2b_d = din("w2b", [H, H])
    b2n_d = din("b2n", [H, 1])     # 63*b2b/63.000001
    w3a_d = din("w3a", [H, H])
    b3a_d = din("b3a", [H, 1])
    w3b_d = din("w3b", [H, H])
    b3b_d = din("b3b", [H, 1])
    w4s_d = din("w4s", [H, H])
    w4r_d = din("w4r", [H, H])
    w2bk_d = din("w2bk", [H, H])   # w2b @ w4a_k
    b4a_d = din("b4a", [H, 1])     # b4a + b2b @ w4a_k
    w4o_d = din("w4o", [H, N_OUT])  # w4b @ wout (bias added on host)

    # packed output: per item [128, 3*512]; rows 32k:32k+16 of group g = chunk 3g+k
    y_d = nc.dram_tensor("y", [B_LOC, N_OUT, N_EDGES], F32, kind="ExternalOutput").ap()

    scale2n = 1.0 / (63.0 + 1e-6)

    with tile.TileContext(nc) as tc:
        with (
            tc.tile_pool(name="wp", bufs=1) as wp,
            tc.tile_pool(name="hp", bufs=3) as hp,
            tc.tile_pool(name="h2p", bufs=2) as h2p,
            tc.tile_pool(name="h4p", bufs=6) as h4p,
            tc.tile_pool(name="op", bufs=2) as op,
            tc.tile_pool(name="pbig", bufs=2, space="PSUM") as pbig,   # [128,1024]
            tc.tile_pool(name="ppo", bufs=2, space="PSUM") as ppo,     # [16,512]
            tc.tile_pool(name="psml", bufs=2, space="PSUM") as psml,   # [128,512]
        ):
            # ---- resident weights (DMA f32, round to f32r on DVE once) ----
            def wload(d, shape):
                raw = wp.tile(shape, F32, tag=d.tensor.name + "_raw")
                nc.sync.dma_start(raw, d)
                t = wp.tile(shape, F32R, tag=d.tensor.name + "_sb")
                nc.vector.tensor_copy(t, raw)
                return t

            w1a = wload(w1a_d, [N_IN, H])
            w1b = wload(w1b_d, [H, H])
            w2s = wload(w2s_d, [H, H])
            w2r = wload(w2r_d, [H, H])
            w2b = wload(w2b_d, [H, H])
            w3a = wload(w3a_d, [H, H])
            w3b = wload(w3b_d, [H, H])
            w4s = wload(w4s_d, [H, H])
            w4r = wload(w4r_d, [H, H])
            w2bk = wload(w2bk_d, [H, H])
            w4o = wload(w4o_d, [H, N_OUT])
            biases = {}
            for d in (b1a_d, b1b_d, b2a_d, b2n_d, b3a_d, b3b_d, b4a_d):
                t = wp.tile([H, 1], F32, tag=d.tensor.name + "_sb")
                nc.sync.dma_start(t, d)
                biases[d.tensor.name] = t

            xTraw = wp.tile([N_IN, B_LOC * N_NODES], F32)
            nc.sync.dma_start(xTraw, xT_d)
            xT = wp.tile([N_IN, B_LOC * N_NODES], F32R)
            nc.gpsimd.tensor_copy(xT, xTraw)

            # ---- MLP1 over all 512 tokens ----
            p1 = psml.tile([H, B_LOC * N_NODES], F32, tag="psml")
            nc.tensor.matmul(p1, w1a, xT, start=True, stop=True)
            h1T = hp.tile([H, B_LOC * N_NODES], F32R, tag="h1T")
            nc.scalar.activation(h1T, p1, _AF.Relu, bias=biases["b1a"])
            p2 = psml.tile([H, B_LOC * N_NODES], F32, tag="psml")
            nc.tensor.matmul(p2, w1b, h1T, start=True, stop=True)
            hT = hp.tile([H, B_LOC * N_NODES], F32, tag="hT")
            nc.scalar.activation(hT, p2, _AF.Identity, bias=biases["b1b"])

            state = {}  # per-item tiles for the delayed MLP4 stage

            def phase_a(b):
                """MLP2 first layer + edge2node + MLP3 for item b."""
                hT9 = hp.tile([H, 9 * N_NODES], F32R, tag="hT9")
                nc.gpsimd.tensor_copy(
                    hT9, _ap(hT, b * N_NODES, [[0, 9], [1, N_NODES]])
                )
                h2T = h2p.tile([H, N_EDGES], F32R, tag="h2T")
                for p in range(4):
                    cs = (2 * p, 2 * p + 1)
                    pmm = pbig.tile([H, 1024], F32, tag="pbig")
                    for k, c in enumerate(cs):
                        d0, nb = CHUNKS[c]
                        sl = slice(512 * k, 512 * (k + 1))
                        nc.tensor.matmul(
                            pmm[:, sl], w2s,
                            _ap(hT9, d0, [[65, nb], [1, N_NODES]]),
                            start=True, stop=False,
                        )
                        nc.tensor.matmul(
                            pmm[:, sl], w2r, hT9[:, 0:512],
                            start=False, stop=True,
                        )
                    if p < 3:
                        nc.scalar.activation(
                            h2T[:, 1024 * p:1024 * (p + 1)], pmm, _AF.Relu,
                            bias=biases["b2a"],
                        )
                    else:
                        # chunk 7 overlaps chunk 6 by one block: split the evac
                        nc.scalar.activation(
                            h2T[:, 3072:3584], pmm[:, 0:512], _AF.Relu,
                            bias=biases["b2a"],
                        )
                        nc.scalar.activation(
                            h2T[:, 3520:4032], pmm[:, 512:1024], _AF.Relu,
                            bias=biases["b2a"],
                        )
                # edge2node: S[j] = sum_d h2T[:, (d-1)*64 + j]
                S = hp.tile([H, N_NODES], F32R, tag="S")
                with nc.allow_low_precision(reason="f32r edge2node reduction"):
                    nc.vector.tensor_reduce(
                        S, _ap(h2T, 0, [[1, N_NODES], [N_NODES, 63]]),
                        axis=mybir.AxisListType.X, op=_ALU.add,
                    )
                pagg = psml.tile([H, N_NODES], F32, tag="psml")
                nc.tensor.matmul(pagg, w2b, S, start=True, stop=True)
                aggT = hp.tile([H, N_NODES], F32R, tag="aggT")
                nc.scalar.activation(
                    aggT, pagg, _AF.Identity, bias=biases["b2n"], scale=scale2n
                )
                pn1 = psml.tile([H, N_NODES], F32, tag="psml")
                nc.tensor.matmul(pn1, w3a, aggT, start=True, stop=True)
                n1T = hp.tile([H, N_NODES], F32R, tag="n1T")
                nc.scalar.activation(n1T, pn1, _AF.Relu, bias=biases["b3a"])
                pn2 = psml.tile([H, N_NODES], F32, tag="psml")
                nc.tensor.matmul(pn2, w3b, n1T, start=True, stop=True)
                nT = hp.tile([H, N_NODES], F32, tag="nT")
                nc.scalar.activation(nT, pn2, _AF.Identity, bias=biases["b3b"])
                nT9 = hp.tile([H, 9 * N_NODES], F32R, tag="nT9")
                nc.gpsimd.tensor_copy(nT9, _ap(nT, 0, [[0, 9], [1, N_NODES]]))
                state[b] = (h2T, nT9)

            def phase_b(b):
                """MLP4 + folded output layer for item b."""
                h2T, nT9 = state.pop(b)
                outP = op.tile([N_OUT, N_EDGES], F32, tag="outP")
                h4s = []
                for p in range(4):
                    cs = (2 * p, 2 * p + 1)
                    pm4 = pbig.tile([H, 1024], F32, tag="pbig")
                    for k, c in enumerate(cs):
                        d0, nb = CHUNKS[c]
                        sl = slice(512 * k, 512 * (k + 1))
                        e0 = (d0 - 1) * N_NODES
                        nc.tensor.matmul(
                            pm4[:, sl], w4s,
                            _ap(nT9, d0, [[65, nb], [1, N_NODES]]),
                            start=True, stop=False,
                        )
                        nc.tensor.matmul(
                            pm4[:, sl], w4r, nT9[:, 0:512],
                            start=False, stop=False,
                        )
                        nc.tensor.matmul(
                            pm4[:, sl], w2bk, h2T[:, e0:e0 + 512],
                            start=False, stop=True,
                        )
                    h4T = h4p.tile([H, 1024], F32R, tag="h4T")
                    if p < 2:
                        nc.scalar.activation(
                            h4T, pm4, _AF.Relu, bias=biases["b4a"],
                        )
                    else:
                        nc.vector.tensor_scalar(
                            h4T, pm4, biases["b4a"], 0.0, _ALU.add, _ALU.max,
                        )
                    h4s.append(h4T)
                # output layer: one [16,512] matmul per chunk; evacuations
                # split between ACT and DVE
                for c in range(8):
                    h4T = h4s[c // 2]
                    hsl = slice(512 * (c % 2), 512 * (c % 2 + 1))
                    po = ppo.tile([N_OUT, 512], F32, tag="ppo")
                    nc.tensor.matmul(po, w4o, h4T[:, hsl], start=True, stop=True)
                    s0 = 512 * c if c < 7 else 3520
                    if c % 2 == 0:
                        nc.scalar.activation(
                            outP[:, s0:s0 + 512], po, _AF.Identity, bias=0.0
                        )
                    else:
                        nc.vector.tensor_copy(outP[:, s0:s0 + 512], po)
                nc.sync.dma_start(y_d[b], outP)

            for b in range(B_LOC):
                phase_a(b)
                if b >= 1:
                    phase_b(b - 1)
            phase_b(B_LOC - 1)

    nc.compile()
    return nc


_CACHE = {}


def _get_nc():
    if "nc" not in _CACHE:
        _CACHE["nc"] = build_kernel()
        _CACHE["perm"] = _edge_perm()
    return _CACHE["nc"], _CACHE["perm"]


def make_in_maps(inputs):
    w2b = np.asarray(inputs["w2b"], np.float32)
    w4a = np.asarray(inputs["w4a"], np.float32)
    b2b = np.asarray(inputs["b2b"], np.float32)
    w4a_k = w4a[2 * H:]
    common = {
        "w1a": inputs["w1a"],
        "b1a": inputs["b1a"][:, None],
        "w1b": inputs["w1b"],
        "b1b": inputs["b1b"][:, None],
        "w2s": inputs["w2a"][:H],
        "w2r": inputs["w2a"][H:],
        "b2a": inputs["b2a"][:, None],
        "w2b": w2b,
        "b2n": (63.0 * b2b / (63.0 + 1e-6))[:, None],
        "w3a": inputs["w3a"],
        "b3a": inputs["b3a"][:, None],
        "w3b": inputs["w3b"],
        "b3b": inputs["b3b"][:, None],
        "w4s": w4a[:H],
        "w4r": w4a[H:2 * H],
        "w2bk": w2b @ w4a_k,
        "b4a": (np.asarray(inputs["b4a"], np.float32) + b2b @ w4a_k)[:, None],
        "w4o": np.asarray(inputs["w4b"], np.float32) @ inputs["wout"],
    }
    common = {k: np.ascontiguousarray(v, np.float32) for k, v in common.items()}
    x = np.ascontiguousarray(inputs["x"], np.float32)
    in_maps = []
    for c in range(N_CORES):
        xs = x[c * B_LOC:(c + 1) * B_LOC]
        xT = np.ascontiguousarray(xs.reshape(B_LOC * N_NODES, N_IN).T)
        in_maps.append({**common, "xT": xT})
    return in_maps


def gather_out(results, perm, inputs):
    b4o = (np.asarray(inputs["b4b"], np.float32) @ inputs["wout"]
           + inputs["bout"]).astype(np.float32)  # [16]
    inv = np.empty_like(perm)
    inv[perm] = np.arange(N_EDGES)
    out = np.empty((BATCH, N_EDGES, N_OUT), np.float32)
    for cr in range(N_CORES):
        y = np.asarray(results[cr]["y"])  # [B_LOC, 16, 4032]
        full = y.transpose(0, 2, 1) + b4o  # [B_LOC, E'', 16]
        out[cr * B_LOC:(cr + 1) * B_LOC] = full[:, inv, :]
    return out


def kernel(**inputs):
    nc, perm = _get_nc()
    in_maps = make_in_maps(inputs)
    res = run_bass_kernel_spmd(nc, in_maps, core_ids=list(range(N_CORES)))
    return gather_out(res.results, perm, inputs)


# revision 10
# speedup vs baseline: 1.8924x; 1.1861x over previous
"""Trainium2 Bass kernel for nn_Encoder (NRI-style GNN message-passing encoder).

Reference math:
  h  = MLP1(x)                       [B,N,H]   N=64 nodes, H=128
  e  = MLP2(node2edge(h))            [B,E,H]   E=4032 edges (fully connected)
  n  = MLP3(edge2node(e))            [B,N,H]
  e2 = MLP4([node2edge(n), e])       [B,E,H]
  out= e2 @ wout + bout              [B,E,16]

Distribution: data-parallel over batch, 8 items per core x 8 cores.

Kernel restructuring (validated in numpy + CoreSim):
- Feature-major activations [features(partition), tokens(free)]; weights
  [fan_in, fan_out] used directly as PE stationary (lhsT).
- Edge reorder (receiver-major): e'' = (d-1)*64 + j  <=>  edge
  (sender=(j-d)%64, receiver=j), d=1..63. With hT9 = tile(hT_b, 9):
    receivers chunk: hT9[:, 0:ncols]                  (contiguous)
    senders  chunk: offset 64-d0, AP [[63,nb],[1,64]] (shifted windows)
  so node2edge is free, folded into matmul access patterns.
- edge2node = plain strided reduce on h2T: S[j] = sum_d h2T[:, (d-1)*64+j]
  (one DVE tensor_reduce per item, no materialization).
- x_skip eliminated: e appears only as MLP4's third K-block, so
  W2bk = w2b @ w4a_k and b4a' = b4a + b2b @ w4a_k are folded host-side and
  MLP4 reads h2T directly (exact algebra, fewer roundings).
- Final linear folded: w4o = w4b @ wout; its bias b4o is added on HOST.
- MLP4-out chunks packed 3-per-PSUM-bank at base partitions 0/32/64.
- float32r (fast PE mode) everywhere; f32r operands produced by rounding
  compute instructions as the compiler requires.

The harness calls kernel(**inputs) with full unsharded inputs.
"""
import sys

sys.path.insert(0, "/opt/trn_rl_repo")

import numpy as np

import concourse.bass as bass
from concourse import bacc
import concourse.mybir as mybir
import concourse.tile as tile
from concourse.bass_utils import run_bass_kernel_spmd

F32 = mybir.dt.float32
F32R = mybir.dt.float32r

N_NODES = 64
N_EDGES = 4032
BATCH = 64
N_IN = 64
H = 128
N_OUT = 16
N_CORES = 8
B_LOC = BATCH // N_CORES

# 63 s-blocks (s = 1..63; block s holds edges (sender=(j+s)%64, receiver=j))
# -> 8 uniform chunks of 8 blocks; the last chunk starts at s=56, overlapping
# chunk 6 by one block so every chunk is exactly 512 columns.
CHUNKS = [(1 + 8 * c, 8) for c in range(7)] + [(56, 8)]

_AF = mybir.ActivationFunctionType
_ALU = mybir.AluOpType


def _edge_perm():
    """perm[p] = original edge index of reordered edge p = (s-1)*64 + j,
    which is edge (sender=(j+s)%64, receiver=j)."""
    s, j = np.meshgrid(np.arange(1, 64), np.arange(64), indexing="ij")
    i = (j + s) % 64
    return (i * 63 + (j - (j > i))).reshape(-1)


def _ap(t, off, dims):
    return bass.AP(tensor=t.tensor, offset=t.offset + off, ap=[t.ap[0]] + dims)


def build_kernel():
    nc = bacc.Bacc("TRN2", target_bir_lowering=False, debug=False)

    def din(name, shape):
        return nc.dram_tensor(name, shape, F32, kind="ExternalInput").ap()

    xT_d = din("xT", [N_IN, B_LOC * N_NODES])
    w1a_d = din("w1a", [N_IN, H])
    b1a_d = din("b1a", [H, 1])
    w1b_d = din("w1b", [H, H])
    b1b_d = din("b1b", [H, 1])
    w2s_d = din("w2s", [H, H])
    w2r_d = din("w2r", [H, H])
    b2a_d = din("b2a", [H, 1])
    w2b_d = din("w2b", [H, H])
    b2n_d = din("b2n", [H, 1])     # 63*b2b/63.000001
    w3a_d = din("w3a", [H, H])
    b3a_d = din("b3a", [H, 1])
    w3b_d = din("w3b", [H, H])
    b3b_d = din("b3b", [H, 1])
    w4s_d = din("w4s", [H, H])
    w4r_d = din("w4r", [H, H])
    w2bk_d = din("w2bk", [H, H])   # w2b @ w4a_k
    b4a_d = din("b4a", [H, 1])     # b4a + b2b @ w4a_k
    w4o_d = din("w4o", [H, N_OUT])  # w4b @ wout (bias added on host)

    # packed output: per item [128, 3*512]; rows 32k:32k+16 of group g = chunk 3g+k
    y_d = nc.dram_tensor("y", [B_LOC, N_OUT, N_EDGES], F32, kind="ExternalOutput").ap()

    scale2n = 1.0 / (63.0 + 1e-6)

    with tile.TileContext(nc) as tc:
        with (
            tc.tile_pool(name="wp", bufs=1) as wp,
            tc.tile_pool(name="hp", bufs=4) as hp,
            tc.tile_pool(name="h2p", bufs=3) as h2p,
            tc.tile_pool(name="h4p", bufs=6) as h4p,
            tc.tile_pool(name="op", bufs=3) as op,
            tc.tile_pool(name="pbig", bufs=2, space="PSUM") as pbig,   # [128,1024]
            tc.tile_pool(name="ppo", bufs=2, space="PSUM") as ppo,     # [16,512]
            tc.tile_pool(name="psml", bufs=2, space="PSUM") as psml,   # [128,512]
        ):
            # ---- resident weights (DMA f32, round to f32r on DVE once) ----
            def wload(d, shape):
                raw = wp.tile(shape, F32, tag=d.tensor.name + "_raw")
                nc.sync.dma_start(raw, d)
                t = wp.tile(shape, F32R, tag=d.tensor.name + "_sb")
                nc.vector.tensor_copy(t, raw)
                return t

            w1a = wload(w1a_d, [N_IN, H])
            w1b = wload(w1b_d, [H, H])
            w2s = wload(w2s_d, [H, H])
            w2r = wload(w2r_d, [H, H])
            w2b = wload(w2b_d, [H, H])
            w3a = wload(w3a_d, [H, H])
            w3b = wload(w3b_d, [H, H])
            w4s = wload(w4s_d, [H, H])
            w4r = wload(w4r_d, [H, H])
            w2bk = wload(w2bk_d, [H, H])
            w4o = wload(w4o_d, [H, N_OUT])
            biases = {}
            for d in (b1a_d, b1b_d, b2a_d, b2n_d, b3a_d, b3b_d, b4a_d):
                t = wp.tile([H, 1], F32, tag=d.tensor.name + "_sb")
                nc.sync.dma_start(t, d)
                biases[d.tensor.name] = t

            xTraw = wp.tile([N_IN, B_LOC * N_NODES], F32)
            nc.sync.dma_start(xTraw, xT_d)
            xT = wp.tile([N_IN, B_LOC * N_NODES], F32R)
            nc.gpsimd.tensor_copy(xT, xTraw)

            # ---- MLP1 over all 512 tokens ----
            p1 = psml.tile([H, B_LOC * N_NODES], F32, tag="psml")
            nc.tensor.matmul(p1, w1a, xT, start=True, stop=True)
            h1T = hp.tile([H, B_LOC * N_NODES], F32R, tag="h1T")
            nc.scalar.activation(h1T, p1, _AF.Relu, bias=biases["b1a"])
            p2 = psml.tile([H, B_LOC * N_NODES], F32, tag="psml")
            nc.tensor.matmul(p2, w1b, h1T, start=True, stop=True)
            hT = hp.tile([H, B_LOC * N_NODES], F32, tag="hT")
            nc.scalar.activation(hT, p2, _AF.Identity, bias=biases["b1b"])

            state = {}  # per-item tiles for the delayed MLP4 stage

            def phase_a(b):
                """MLP2 first layer + edge2node + MLP3 for item b."""
                hT9 = hp.tile([H, 9 * N_NODES], F32R, tag="hT9")
                nc.vector.tensor_copy(
                    hT9, _ap(hT, b * N_NODES, [[0, 9], [1, N_NODES]])
                )
                h2T = h2p.tile([H, N_EDGES], F32R, tag="h2T")
                for p in range(4):
                    cs = (2 * p, 2 * p + 1)
                    pmm = pbig.tile([H, 1024], F32, tag="pbig")
                    for k, c in enumerate(cs):
                        d0, nb = CHUNKS[c]
                        sl = slice(512 * k, 512 * (k + 1))
                        nc.tensor.matmul(
                            pmm[:, sl], w2s,
                            _ap(hT9, d0, [[65, nb], [1, N_NODES]]),
                            start=True, stop=False,
                        )
                        nc.tensor.matmul(
                            pmm[:, sl], w2r, hT9[:, 0:512],
                            start=False, stop=True,
                        )
                    if p < 3:
                        nc.scalar.activation(
                            h2T[:, 1024 * p:1024 * (p + 1)], pmm, _AF.Relu,
                            bias=biases["b2a"],
                        )
                    else:
                        # chunk 7 overlaps chunk 6 by one block: split the evac
                        nc.scalar.activation(
                            h2T[:, 3072:3584], pmm[:, 0:512], _AF.Relu,
                            bias=biases["b2a"],
                        )
                        nc.scalar.activation(
                            h2T[:, 3520:4032], pmm[:, 512:1024], _AF.Relu,
                            bias=biases["b2a"],
                        )
                # edge2node: S[j] = sum_s h2T[:, (s-1)*64 + j] via a
                # contiguous halving add-tree (DVE streams at full rate).
                A = hp.tile([H, 2048], F32R, tag="treeA")
                nc.vector.tensor_add(
                    A[:, 0:1984], h2T[:, 0:1984], h2T[:, 2048:4032]
                )
                nc.vector.tensor_copy(A[:, 1984:2048], h2T[:, 1984:2048])
                w = 1024
                while w >= N_NODES:
                    nc.vector.tensor_add(A[:, 0:w], A[:, 0:w], A[:, w:2 * w])
                    w //= 2
                S = A[:, 0:N_NODES]
                pagg = psml.tile([H, N_NODES], F32, tag="psml")
                nc.tensor.matmul(pagg, w2b, S, start=True, stop=True)
                aggT = hp.tile([H, N_NODES], F32R, tag="aggT")
                nc.scalar.activation(
                    aggT, pagg, _AF.Identity, bias=biases["b2n"], scale=scale2n
                )
                pn1 = psml.tile([H, N_NODES], F32, tag="psml")
                nc.tensor.matmul(pn1, w3a, aggT, start=True, stop=True)
                n1T = hp.tile([H, N_NODES], F32R, tag="n1T")
                nc.scalar.activation(n1T, pn1, _AF.Relu, bias=biases["b3a"])
                pn2 = psml.tile([H, N_NODES], F32, tag="psml")
                nc.tensor.matmul(pn2, w3b, n1T, start=True, stop=True)
                nT = hp.tile([H, N_NODES], F32, tag="nT")
                nc.scalar.activation(nT, pn2, _AF.Identity, bias=biases["b3b"])
                nT9 = hp.tile([H, 9 * N_NODES], F32R, tag="nT9")
                nc.scalar.activation(nT9, _ap(nT, 0, [[0, 9], [1, N_NODES]]), _AF.Identity, bias=0.0)
                state[b] = (h2T, nT9)

            def phase_b(b):
                """MLP4 + folded output layer for item b."""
                h2T, nT9 = state.pop(b)
                outP = op.tile([N_OUT, N_EDGES], F32, tag="outP")
                h4s = []
                for p in range(4):
                    cs = (2 * p, 2 * p + 1)
                    pm4 = pbig.tile([H, 1024], F32, tag="pbig")
                    for k, c in enumerate(cs):
                        d0, nb = CHUNKS[c]
                        sl = slice(512 * k, 512 * (k + 1))
                        e0 = (d0 - 1) * N_NODES
                        nc.tensor.matmul(
                            pm4[:, sl], w4s,
                            _ap(nT9, d0, [[65, nb], [1, N_NODES]]),
                            start=True, stop=False,
                        )
                        nc.tensor.matmul(
                            pm4[:, sl], w4r, nT9[:, 0:512],
                            start=False, stop=False,
                        )
                        nc.tensor.matmul(
                            pm4[:, sl], w2bk, h2T[:, e0:e0 + 512],
                            start=False, stop=True,
                        )
                    h4T = h4p.tile([H, 1024], F32R, tag="h4T")
                    if p < 2:
                        nc.scalar.activation(
                            h4T, pm4, _AF.Relu, bias=biases["b4a"],
                        )
                    else:
                        nc.vector.tensor_scalar(
                            h4T, pm4, biases["b4a"], 0.0, _ALU.add, _ALU.max,
                        )
                    h4s.append(h4T)
                # output layer: one [16,512] matmul per chunk; evacuations
                # split between ACT and DVE
                for c in range(8):
                    h4T = h4s[c // 2]
                    hsl = slice(512 * (c % 2), 512 * (c % 2 + 1))
                    po = ppo.tile([N_OUT, 512], F32, tag="ppo")
                    nc.tensor.matmul(po, w4o, h4T[:, hsl], start=True, stop=True)
                    s0 = 512 * c if c < 7 else 3520
                    if c % 2 == 0:
                        nc.scalar.activation(
                            outP[:, s0:s0 + 512], po, _AF.Identity, bias=0.0
                        )
                    else:
                        nc.vector.tensor_copy(outP[:, s0:s0 + 512], po)
                nc.sync.dma_start(y_d[b], outP)

            for b in range(B_LOC):
                phase_a(b)
                if b >= 2:
                    phase_b(b - 2)
            phase_b(B_LOC - 2)
            phase_b(B_LOC - 1)

    nc.compile()
    return nc


_CACHE = {}


def _get_nc():
    if "nc" not in _CACHE:
        _CACHE["nc"] = build_kernel()
        _CACHE["perm"] = _edge_perm()
    return _CACHE["nc"], _CACHE["perm"]


def make_in_maps(inputs):
    w2b = np.asarray(inputs["w2b"], np.float32)
    w4a = np.asarray(inputs["w4a"], np.float32)
    b2b = np.asarray(inputs["b2b"], np.float32)
    w4a_k = w4a[2 * H:]
    common = {
        "w1a": inputs["w1a"],
        "b1a": inputs["b1a"][:, None],
        "w1b": inputs["w1b"],
        "b1b": inputs["b1b"][:, None],
        "w2s": inputs["w2a"][:H],
        "w2r": inputs["w2a"][H:],
        "b2a": inputs["b2a"][:, None],
        "w2b": w2b,
        "b2n": (63.0 * b2b / (63.0 + 1e-6))[:, None],
        "w3a": inputs["w3a"],
        "b3a": inputs["b3a"][:, None],
        "w3b": inputs["w3b"],
        "b3b": inputs["b3b"][:, None],
        "w4s": w4a[:H],
        "w4r": w4a[H:2 * H],
        "w2bk": w2b @ w4a_k,
        "b4a": (np.asarray(inputs["b4a"], np.float32) + b2b @ w4a_k)[:, None],
        "w4o": np.asarray(inputs["w4b"], np.float32) @ inputs["wout"],
    }
    common = {k: np.ascontiguousarray(v, np.float32) for k, v in common.items()}
    x = np.ascontiguousarray(inputs["x"], np.float32)
    in_maps = []
    for c in range(N_CORES):
        xs = x[c * B_LOC:(c + 1) * B_LOC]
        xT = np.ascontiguousarray(xs.reshape(B_LOC * N_NODES, N_IN).T)
        in_maps.append({**common, "xT": xT})
    return in_maps


def gather_out(results, perm, inputs):
    b4o = (np.asarray(inputs["b4b"], np.float32) @ inputs["wout"]
           + inputs["bout"]).astype(np.float32)  # [16]
    inv = np.empty_like(perm)
    inv[perm] = np.arange(N_EDGES)
    out = np.empty((BATCH, N_EDGES, N_OUT), np.float32)
    for cr in range(N_CORES):
        y = np.asarray(results[cr]["y"])  # [B_LOC, 16, 4032]
        full = y.transpose(0, 2, 1) + b4o  # [B_LOC, E'', 16]
        out[cr * B_LOC:(cr + 1) * B_LOC] = full[:, inv, :]
    return out


def kernel(**inputs):
    nc, perm = _get_nc()
    in_maps = make_in_maps(inputs)
    res = run_bass_kernel_spmd(nc, in_maps, core_ids=list(range(N_CORES)))
    return gather_out(res.results, perm, inputs)


# revision 11
# speedup vs baseline: 1.9619x; 1.0367x over previous
"""Trainium2 Bass kernel for nn_Encoder (NRI-style GNN message-passing encoder).

Reference math:
  h  = MLP1(x)                       [B,N,H]   N=64 nodes, H=128
  e  = MLP2(node2edge(h))            [B,E,H]   E=4032 edges (fully connected)
  n  = MLP3(edge2node(e))            [B,N,H]
  e2 = MLP4([node2edge(n), e])       [B,E,H]
  out= e2 @ wout + bout              [B,E,16]

Distribution: data-parallel over batch, 8 items per core x 8 cores.

Kernel restructuring (validated in numpy + CoreSim):
- Feature-major activations [features(partition), tokens(free)]; weights
  [fan_in, fan_out] used directly as PE stationary (lhsT).
- Edge reorder (receiver-major): e'' = (d-1)*64 + j  <=>  edge
  (sender=(j-d)%64, receiver=j), d=1..63. With hT9 = tile(hT_b, 9):
    receivers chunk: hT9[:, 0:ncols]                  (contiguous)
    senders  chunk: offset 64-d0, AP [[63,nb],[1,64]] (shifted windows)
  so node2edge is free, folded into matmul access patterns.
- edge2node = plain strided reduce on h2T: S[j] = sum_d h2T[:, (d-1)*64+j]
  (one DVE tensor_reduce per item, no materialization).
- x_skip eliminated: e appears only as MLP4's third K-block, so
  W2bk = w2b @ w4a_k and b4a' = b4a + b2b @ w4a_k are folded host-side and
  MLP4 reads h2T directly (exact algebra, fewer roundings).
- Final linear folded: w4o = w4b @ wout; its bias b4o is added on HOST.
- MLP4-out chunks packed 3-per-PSUM-bank at base partitions 0/32/64.
- float32r (fast PE mode) everywhere; f32r operands produced by rounding
  compute instructions as the compiler requires.

The harness calls kernel(**inputs) with full unsharded inputs.
"""
import sys

sys.path.insert(0, "/opt/trn_rl_repo")

import numpy as np

import concourse.bass as bass
from concourse import bacc
import concourse.mybir as mybir
import concourse.tile as tile
from concourse.bass_utils import run_bass_kernel_spmd

F32 = mybir.dt.float32
F32R = mybir.dt.float32r

N_NODES = 64
N_EDGES = 4032
BATCH = 64
N_IN = 64
H = 128
N_OUT = 16
N_CORES = 8
B_LOC = BATCH // N_CORES

# 63 s-blocks (s = 1..63; block s holds edges (sender=(j+s)%64, receiver=j))
# -> 8 uniform chunks of 8 blocks; the last chunk starts at s=56, overlapping
# chunk 6 by one block so every chunk is exactly 512 columns.
CHUNKS = [(1 + 8 * c, 8) for c in range(7)] + [(56, 8)]

_AF = mybir.ActivationFunctionType
_ALU = mybir.AluOpType


def _edge_perm():
    """perm[p] = original edge index of reordered edge p = (s-1)*64 + j,
    which is edge (sender=(j+s)%64, receiver=j)."""
    s, j = np.meshgrid(np.arange(1, 64), np.arange(64), indexing="ij")
    i = (j + s) % 64
    return (i * 63 + (j - (j > i))).reshape(-1)


def _ap(t, off, dims):
    return bass.AP(tensor=t.tensor, offset=t.offset + off, ap=[t.ap[0]] + dims)


def build_kernel():
    nc = bacc.Bacc("TRN2", target_bir_lowering=False, debug=False)

    def din(name, shape):
        return nc.dram_tensor(name, shape, F32, kind="ExternalInput").ap()

    xT_d = din("xT", [N_IN, B_LOC * N_NODES])
    w1a_d = din("w1a", [N_IN, H])
    b1a_d = din("b1a", [H, 1])
    w1b_d = din("w1b", [H, H])
    b1b_d = din("b1b", [H, 1])
    w2s_d = din("w2s", [H, H])
    w2r_d = din("w2r", [H, H])
    b2a_d = din("b2a", [H, 1])
    w2b_d = din("w2b", [H, H])
    b2n_d = din("b2n", [H, 1])     # 63*b2b/63.000001
    w3a_d = din("w3a", [H, H])
    b3a_d = din("b3a", [H, 1])
    w3b_d = din("w3b", [H, H])
    b3b_d = din("b3b", [H, 1])
    w4s_d = din("w4s", [H, H])
    w4r_d = din("w4r", [H, H])
    w2bk_d = din("w2bk", [H, H])   # w2b @ w4a_k
    b4a_d = din("b4a", [H, 1])     # b4a + b2b @ w4a_k
    w4o_d = din("w4o", [H, N_OUT])  # w4b @ wout (bias added on host)

    # packed output: per item [128, 3*512]; rows 32k:32k+16 of group g = chunk 3g+k
    y_d = nc.dram_tensor("y", [B_LOC, N_OUT, N_EDGES], F32, kind="ExternalOutput").ap()

    scale2n = 1.0 / (63.0 + 1e-6)

    with tile.TileContext(nc) as tc:
        with (
            tc.tile_pool(name="wp", bufs=1) as wp,
            tc.tile_pool(name="hp", bufs=4) as hp,
            tc.tile_pool(name="h2p", bufs=3) as h2p,
            tc.tile_pool(name="h4p", bufs=6) as h4p,
            tc.tile_pool(name="op", bufs=3) as op,
            tc.tile_pool(name="pbig", bufs=2, space="PSUM") as pbig,   # [128,1024]
            tc.tile_pool(name="ppo", bufs=3, space="PSUM") as ppo,     # [16,512]
            tc.tile_pool(name="psml", bufs=1, space="PSUM") as psml,   # [128,512]
        ):
            # ---- resident weights (DMA f32, round to f32r on DVE once) ----
            def wload(d, shape):
                raw = wp.tile(shape, F32, tag=d.tensor.name + "_raw")
                nc.sync.dma_start(raw, d)
                t = wp.tile(shape, F32R, tag=d.tensor.name + "_sb")
                nc.vector.tensor_copy(t, raw)
                return t

            w1a = wload(w1a_d, [N_IN, H])
            w1b = wload(w1b_d, [H, H])
            w2s = wload(w2s_d, [H, H])
            w2r = wload(w2r_d, [H, H])
            w2b = wload(w2b_d, [H, H])
            w3a = wload(w3a_d, [H, H])
            w3b = wload(w3b_d, [H, H])
            w4s = wload(w4s_d, [H, H])
            w4r = wload(w4r_d, [H, H])
            w2bk = wload(w2bk_d, [H, H])
            w4o = wload(w4o_d, [H, N_OUT])
            biases = {}
            for d in (b1a_d, b1b_d, b2a_d, b2n_d, b3a_d, b3b_d, b4a_d):
                t = wp.tile([H, 1], F32, tag=d.tensor.name + "_sb")
                nc.sync.dma_start(t, d)
                biases[d.tensor.name] = t

            xTraw = wp.tile([N_IN, B_LOC * N_NODES], F32)
            nc.sync.dma_start(xTraw, xT_d)
            xT = wp.tile([N_IN, B_LOC * N_NODES], F32R)
            nc.vector.tensor_copy(xT, xTraw)

            # ---- MLP1 over all 512 tokens ----
            p1 = psml.tile([H, B_LOC * N_NODES], F32, tag="psml")
            nc.tensor.matmul(p1, w1a, xT, start=True, stop=True)
            h1T = hp.tile([H, B_LOC * N_NODES], F32R, tag="h1T")
            nc.scalar.activation(h1T, p1, _AF.Relu, bias=biases["b1a"])
            p2 = psml.tile([H, B_LOC * N_NODES], F32, tag="psml")
            nc.tensor.matmul(p2, w1b, h1T, start=True, stop=True)
            hT = hp.tile([H, B_LOC * N_NODES], F32, tag="hT")
            nc.scalar.activation(hT, p2, _AF.Identity, bias=biases["b1b"])

            state = {}  # per-item tiles for the delayed MLP4 stage

            def phase_a(b):
                """MLP2 first layer + edge2node + MLP3 for item b."""
                hT9 = hp.tile([H, 9 * N_NODES], F32R, tag="hT9")
                nc.vector.tensor_copy(
                    hT9, _ap(hT, b * N_NODES, [[0, 9], [1, N_NODES]])
                )
                h2T = h2p.tile([H, N_EDGES], F32R, tag="h2T")
                for p in range(4):
                    cs = (2 * p, 2 * p + 1)
                    pmm = pbig.tile([H, 1024], F32, tag="pbig")
                    for k, c in enumerate(cs):
                        d0, nb = CHUNKS[c]
                        sl = slice(512 * k, 512 * (k + 1))
                        nc.tensor.matmul(
                            pmm[:, sl], w2s,
                            _ap(hT9, d0, [[65, nb], [1, N_NODES]]),
                            start=True, stop=False,
                        )
                        nc.tensor.matmul(
                            pmm[:, sl], w2r, hT9[:, 0:512],
                            start=False, stop=True,
                        )
                    if p < 3:
                        nc.scalar.activation(
                            h2T[:, 1024 * p:1024 * (p + 1)], pmm, _AF.Relu,
                            bias=biases["b2a"],
                        )
                    else:
                        # chunk 7 overlaps chunk 6 by one block: split the evac
                        nc.scalar.activation(
                            h2T[:, 3072:3584], pmm[:, 0:512], _AF.Relu,
                            bias=biases["b2a"],
                        )
                        nc.scalar.activation(
                            h2T[:, 3520:4032], pmm[:, 512:1024], _AF.Relu,
                            bias=biases["b2a"],
                        )
                # edge2node: S[j] = sum_s h2T[:, (s-1)*64 + j] via a
                # contiguous halving add-tree (DVE streams at full rate).
                A = hp.tile([H, 2048], F32R, tag="treeA")
                nc.vector.tensor_add(
                    A[:, 0:1984], h2T[:, 0:1984], h2T[:, 2048:4032]
                )
                nc.vector.tensor_copy(A[:, 1984:2048], h2T[:, 1984:2048])
                w = 1024
                while w >= N_NODES:
                    nc.vector.tensor_add(A[:, 0:w], A[:, 0:w], A[:, w:2 * w])
                    w //= 2
                S = A[:, 0:N_NODES]
                pagg = psml.tile([H, N_NODES], F32, tag="psml")
                nc.tensor.matmul(pagg, w2b, S, start=True, stop=True)
                aggT = hp.tile([H, N_NODES], F32R, tag="aggT")
                nc.scalar.activation(
                    aggT, pagg, _AF.Identity, bias=biases["b2n"], scale=scale2n
                )
                pn1 = psml.tile([H, N_NODES], F32, tag="psml")
                nc.tensor.matmul(pn1, w3a, aggT, start=True, stop=True)
                n1T = hp.tile([H, N_NODES], F32R, tag="n1T")
                nc.scalar.activation(n1T, pn1, _AF.Relu, bias=biases["b3a"])
                pn2 = psml.tile([H, N_NODES], F32, tag="psml")
                nc.tensor.matmul(pn2, w3b, n1T, start=True, stop=True)
                nT = hp.tile([H, N_NODES], F32, tag="nT")
                nc.scalar.activation(nT, pn2, _AF.Identity, bias=biases["b3b"])
                nT9 = hp.tile([H, 9 * N_NODES], F32R, tag="nT9")
                nc.scalar.activation(nT9, _ap(nT, 0, [[0, 9], [1, N_NODES]]), _AF.Identity, bias=0.0)
                state[b] = (h2T, nT9)

            def phase_b(b):
                """MLP4 + folded output layer for item b."""
                h2T, nT9 = state.pop(b)
                outP = op.tile([N_OUT, N_EDGES], F32, tag="outP")
                h4s = []
                for p in range(4):
                    cs = (2 * p, 2 * p + 1)
                    pm4 = pbig.tile([H, 1024], F32, tag="pbig")
                    for k, c in enumerate(cs):
                        d0, nb = CHUNKS[c]
                        sl = slice(512 * k, 512 * (k + 1))
                        e0 = (d0 - 1) * N_NODES
                        nc.tensor.matmul(
                            pm4[:, sl], w4s,
                            _ap(nT9, d0, [[65, nb], [1, N_NODES]]),
                            start=True, stop=False,
                        )
                        nc.tensor.matmul(
                            pm4[:, sl], w4r, nT9[:, 0:512],
                            start=False, stop=False,
                        )
                        nc.tensor.matmul(
                            pm4[:, sl], w2bk, h2T[:, e0:e0 + 512],
                            start=False, stop=True,
                        )
                    h4T = h4p.tile([H, 1024], F32R, tag="h4T")
                    if p < 2:
                        nc.scalar.activation(
                            h4T, pm4, _AF.Relu, bias=biases["b4a"],
                        )
                    else:
                        nc.vector.tensor_scalar(
                            h4T, pm4, biases["b4a"], 0.0, _ALU.add, _ALU.max,
                        )
                    h4s.append(h4T)
                # output layer: one [16,512] matmul per chunk; evacuations
                # split between ACT and DVE
                for c in range(8):
                    h4T = h4s[c // 2]
                    hsl = slice(512 * (c % 2), 512 * (c % 2 + 1))
                    po = ppo.tile([N_OUT, 512], F32, tag="ppo")
                    nc.tensor.matmul(po, w4o, h4T[:, hsl], start=True, stop=True)
                    s0 = 512 * c if c < 7 else 3520
                    if c % 2 == 0:
                        nc.scalar.activation(
                            outP[:, s0:s0 + 512], po, _AF.Identity, bias=0.0
                        )
                    else:
                        nc.vector.tensor_copy(outP[:, s0:s0 + 512], po)
                nc.sync.dma_start(y_d[b], outP)

            for b in range(B_LOC):
                phase_a(b)
                if b >= 2:
                    phase_b(b - 2)
            phase_b(B_LOC - 2)
            phase_b(B_LOC - 1)

    nc.compile()
    return nc


_CACHE = {}


def _get_nc():
    if "nc" not in _CACHE:
        _CACHE["nc"] = build_kernel()
        _CACHE["perm"] = _edge_perm()
    return _CACHE["nc"], _CACHE["perm"]


def make_in_maps(inputs):
    w2b = np.asarray(inputs["w2b"], np.float32)
    w4a = np.asarray(inputs["w4a"], np.float32)
    b2b = np.asarray(inputs["b2b"], np.float32)
    w4a_k = w4a[2 * H:]
    common = {
        "w1a": inputs["w1a"],
        "b1a": inputs["b1a"][:, None],
        "w1b": inputs["w1b"],
        "b1b": inputs["b1b"][:, None],
        "w2s": inputs["w2a"][:H],
        "w2r": inputs["w2a"][H:],
        "b2a": inputs["b2a"][:, None],
        "w2b": w2b,
        "b2n": (63.0 * b2b / (63.0 + 1e-6))[:, None],
        "w3a": inputs["w3a"],
        "b3a": inputs["b3a"][:, None],
        "w3b": inputs["w3b"],
        "b3b": inputs["b3b"][:, None],
        "w4s": w4a[:H],
        "w4r": w4a[H:2 * H],
        "w2bk": w2b @ w4a_k,
        "b4a": (np.asarray(inputs["b4a"], np.float32) + b2b @ w4a_k)[:, None],
        "w4o": np.asarray(inputs["w4b"], np.float32) @ inputs["wout"],
    }
    common = {k: np.ascontiguousarray(v, np.float32) for k, v in common.items()}
    x = np.ascontiguousarray(inputs["x"], np.float32)
    in_maps = []
    for c in range(N_CORES):
        xs = x[c * B_LOC:(c + 1) * B_LOC]
        xT = np.ascontiguousarray(xs.reshape(B_LOC * N_NODES, N_IN).T)
        in_maps.append({**common, "xT": xT})
    return in_maps


def gather_out(results, perm, inputs):
    b4o = (np.asarray(inputs["b4b"], np.float32) @ inputs["wout"]
           + inputs["bout"]).astype(np.float32)  # [16]
    inv = np.empty_like(perm)
    inv[perm] = np.arange(N_EDGES)
    out = np.empty((BATCH, N_EDGES, N_OUT), np.float32)
    for cr in range(N_CORES):
        y = np.asarray(results[cr]["y"])  # [B_LOC, 16, 4032]
        full = y.transpose(0, 2, 1) + b4o  # [B_LOC, E'', 16]
        out[cr * B_LOC:(cr + 1) * B_LOC] = full[:, inv, :]
    return out


def kernel(**inputs):
    nc, perm = _get_nc()
    in_maps = make_in_maps(inputs)
    res = run_bass_kernel_spmd(nc, in_maps, core_ids=list(range(N_CORES)))
    return gather_out(res.results, perm, inputs)


# revision 12
# speedup vs baseline: 2.0584x; 1.0492x over previous
"""Trainium2 Bass kernel for nn_Encoder (NRI-style GNN message-passing encoder).

Reference math:
  h  = MLP1(x)                       [B,N,H]   N=64 nodes, H=128
  e  = MLP2(node2edge(h))            [B,E,H]   E=4032 edges (fully connected)
  n  = MLP3(edge2node(e))            [B,N,H]
  e2 = MLP4([node2edge(n), e])       [B,E,H]
  out= e2 @ wout + bout              [B,E,16]

Distribution: data-parallel over batch, 8 items per core x 8 cores.

Kernel restructuring (validated in numpy + CoreSim):
- Feature-major activations [features(partition), tokens(free)]; weights
  [fan_in, fan_out] used directly as PE stationary (lhsT).
- Edge reorder (receiver-major): e'' = (d-1)*64 + j  <=>  edge
  (sender=(j-d)%64, receiver=j), d=1..63. With hT9 = tile(hT_b, 9):
    receivers chunk: hT9[:, 0:ncols]                  (contiguous)
    senders  chunk: offset 64-d0, AP [[63,nb],[1,64]] (shifted windows)
  so node2edge is free, folded into matmul access patterns.
- edge2node = plain strided reduce on h2T: S[j] = sum_d h2T[:, (d-1)*64+j]
  (one DVE tensor_reduce per item, no materialization).
- x_skip eliminated: e appears only as MLP4's third K-block, so
  W2bk = w2b @ w4a_k and b4a' = b4a + b2b @ w4a_k are folded host-side and
  MLP4 reads h2T directly (exact algebra, fewer roundings).
- Final linear folded: w4o = w4b @ wout; its bias b4o is added on HOST.
- MLP4-out chunks packed 3-per-PSUM-bank at base partitions 0/32/64.
- float32r (fast PE mode) everywhere; f32r operands produced by rounding
  compute instructions as the compiler requires.

The harness calls kernel(**inputs) with full unsharded inputs.
"""
import sys

sys.path.insert(0, "/opt/trn_rl_repo")

import numpy as np

import concourse.bass as bass
from concourse import bacc
import concourse.mybir as mybir
import concourse.tile as tile
from concourse.bass_utils import run_bass_kernel_spmd

F32 = mybir.dt.float32
F32R = mybir.dt.float32r

N_NODES = 64
N_EDGES = 4032
BATCH = 64
N_IN = 64
H = 128
N_OUT = 16
N_CORES = 8
B_LOC = BATCH // N_CORES

# 63 s-blocks (s = 1..63; block s holds edges (sender=(j+s)%64, receiver=j))
# -> 8 uniform chunks of 8 blocks; the last chunk starts at s=56, overlapping
# chunk 6 by one block so every chunk is exactly 512 columns.
CHUNKS = [(1 + 8 * c, 8) for c in range(7)] + [(56, 8)]

_AF = mybir.ActivationFunctionType
_ALU = mybir.AluOpType


def _edge_perm():
    """perm[p] = original edge index of reordered edge p = (s-1)*64 + j,
    which is edge (sender=(j+s)%64, receiver=j)."""
    s, j = np.meshgrid(np.arange(1, 64), np.arange(64), indexing="ij")
    i = (j + s) % 64
    return (i * 63 + (j - (j > i))).reshape(-1)


def _ap(t, off, dims):
    return bass.AP(tensor=t.tensor, offset=t.offset + off, ap=[t.ap[0]] + dims)


def build_kernel():
    nc = bacc.Bacc("TRN2", target_bir_lowering=False, debug=False)

    def din(name, shape):
        return nc.dram_tensor(name, shape, F32, kind="ExternalInput").ap()

    # all weights + biases + xT packed into one [128, WTOT] array (host-built):
    # 11 weight blocks of 128 cols (w4o padded), 7 bias cols, 512 xT cols.
    WTOT = 11 * H + 7 + B_LOC * N_NODES
    wpack_d = din("wpack", [H, WTOT])

    # packed output: per item [128, 3*512]; rows 32k:32k+16 of group g = chunk 3g+k
    y_d = nc.dram_tensor("y", [B_LOC, N_OUT, N_EDGES], F32, kind="ExternalOutput").ap()

    scale2n = 1.0 / (63.0 + 1e-6)

    with tile.TileContext(nc) as tc:
        with (
            tc.tile_pool(name="wp", bufs=1) as wp,
            tc.tile_pool(name="hp", bufs=4) as hp,
            tc.tile_pool(name="h2p", bufs=3) as h2p,
            tc.tile_pool(name="h4p", bufs=6) as h4p,
            tc.tile_pool(name="op", bufs=3) as op,
            tc.tile_pool(name="pbig", bufs=2, space="PSUM") as pbig,   # [128,1024]
            tc.tile_pool(name="ppo", bufs=3, space="PSUM") as ppo,     # [16,512]
            tc.tile_pool(name="psml", bufs=1, space="PSUM") as psml,   # [128,512]
        ):
            # ---- one DMA for all weights/biases/xT, one f32r rounding pass ----
            wraw = wp.tile([H, 11 * H + 7 + B_LOC * N_NODES], F32)
            nc.sync.dma_start(wraw, wpack_d)
            wall = wp.tile([H, 11 * H + 7 + B_LOC * N_NODES], F32R)
            nc.vector.tensor_copy(wall, wraw)
            _wnames = ["w1a", "w1b", "w2s", "w2r", "w2b", "w3a", "w3b",
                       "w4s", "w4r", "w2bk", "w4o"]
            _w = {n: wall[:, 128 * i:128 * (i + 1)] for i, n in enumerate(_wnames)}
            w1a = _w["w1a"][0:N_IN, :]
            w1b, w2s, w2r, w2b = _w["w1b"], _w["w2s"], _w["w2r"], _w["w2b"]
            w3a, w3b, w4s, w4r = _w["w3a"], _w["w3b"], _w["w4s"], _w["w4r"]
            w2bk = _w["w2bk"]
            w4o = _w["w4o"][:, 0:N_OUT]
            _bnames = ["b1a", "b1b", "b2a", "b2n", "b3a", "b3b", "b4a"]
            # biases must be plain F32 for ACT/DVE scalar operands
            biases = {}
            for i, n in enumerate(_bnames):
                t = wp.tile([H, 1], F32, tag=n + "_sb")
                nc.vector.tensor_copy(t, wraw[:, 11 * H + i:11 * H + i + 1])
                biases[n] = t
            xT = wall[0:N_IN, 11 * H + 7:]

            # ---- MLP1 over all 512 tokens ----
            p1 = psml.tile([H, B_LOC * N_NODES], F32, tag="psml")
            nc.tensor.matmul(p1, w1a, xT, start=True, stop=True)
            h1T = hp.tile([H, B_LOC * N_NODES], F32R, tag="h1T")
            nc.scalar.activation(h1T, p1, _AF.Relu, bias=biases["b1a"])
            p2 = psml.tile([H, B_LOC * N_NODES], F32, tag="psml")
            nc.tensor.matmul(p2, w1b, h1T, start=True, stop=True)
            hT = hp.tile([H, B_LOC * N_NODES], F32, tag="hT")
            nc.scalar.activation(hT, p2, _AF.Identity, bias=biases["b1b"])

            state = {}  # per-item tiles for the delayed MLP4 stage

            def phase_a(b):
                """MLP2 first layer + edge2node + MLP3 for item b."""
                hT9 = hp.tile([H, 9 * N_NODES], F32R, tag="hT9")
                nc.vector.tensor_copy(
                    hT9, _ap(hT, b * N_NODES, [[0, 9], [1, N_NODES]])
                )
                h2T = h2p.tile([H, N_EDGES], F32R, tag="h2T")
                for p in range(4):
                    cs = (2 * p, 2 * p + 1)
                    pmm = pbig.tile([H, 1024], F32, tag="pbig")
                    for k, c in enumerate(cs):
                        d0, nb = CHUNKS[c]
                        sl = slice(512 * k, 512 * (k + 1))
                        nc.tensor.matmul(
                            pmm[:, sl], w2s,
                            _ap(hT9, d0, [[65, nb], [1, N_NODES]]),
                            start=True, stop=False,
                        )
                        nc.tensor.matmul(
                            pmm[:, sl], w2r, hT9[:, 0:512],
                            start=False, stop=True,
                        )
                    if p < 3:
                        nc.scalar.activation(
                            h2T[:, 1024 * p:1024 * (p + 1)], pmm, _AF.Relu,
                            bias=biases["b2a"],
                        )
                    else:
                        # chunk 7 overlaps chunk 6 by one block: split the evac
                        nc.scalar.activation(
                            h2T[:, 3072:3584], pmm[:, 0:512], _AF.Relu,
                            bias=biases["b2a"],
                        )
                        nc.scalar.activation(
                            h2T[:, 3520:4032], pmm[:, 512:1024], _AF.Relu,
                            bias=biases["b2a"],
                        )
                # edge2node: S[j] = sum_s h2T[:, (s-1)*64 + j] via a
                # contiguous halving add-tree (DVE streams at full rate).
                A = hp.tile([H, 2048], F32R, tag="treeA")
                nc.vector.tensor_add(
                    A[:, 0:1984], h2T[:, 0:1984], h2T[:, 2048:4032]
                )
                nc.vector.tensor_copy(A[:, 1984:2048], h2T[:, 1984:2048])
                w = 1024
                while w >= N_NODES:
                    nc.vector.tensor_add(A[:, 0:w], A[:, 0:w], A[:, w:2 * w])
                    w //= 2
                S = A[:, 0:N_NODES]
                pagg = psml.tile([H, N_NODES], F32, tag="psml")
                nc.tensor.matmul(pagg, w2b, S, start=True, stop=True)
                aggT = hp.tile([H, N_NODES], F32R, tag="aggT")
                nc.scalar.activation(
                    aggT, pagg, _AF.Identity, bias=biases["b2n"], scale=scale2n
                )
                pn1 = psml.tile([H, N_NODES], F32, tag="psml")
                nc.tensor.matmul(pn1, w3a, aggT, start=True, stop=True)
                n1T = hp.tile([H, N_NODES], F32R, tag="n1T")
                nc.scalar.activation(n1T, pn1, _AF.Relu, bias=biases["b3a"])
                pn2 = psml.tile([H, N_NODES], F32, tag="psml")
                nc.tensor.matmul(pn2, w3b, n1T, start=True, stop=True)
                nT = hp.tile([H, N_NODES], F32, tag="nT")
                nc.scalar.activation(nT, pn2, _AF.Identity, bias=biases["b3b"])
                nT9 = hp.tile([H, 9 * N_NODES], F32R, tag="nT9")
                nc.scalar.activation(nT9, _ap(nT, 0, [[0, 9], [1, N_NODES]]), _AF.Identity, bias=0.0)
                state[b] = (h2T, nT9)

            def phase_b(b):
                """MLP4 + folded output layer for item b."""
                h2T, nT9 = state.pop(b)
                outP = op.tile([N_OUT, N_EDGES], F32, tag="outP")
                h4s = []
                for p in range(4):
                    cs = (2 * p, 2 * p + 1)
                    pm4 = pbig.tile([H, 1024], F32, tag="pbig")
                    for k, c in enumerate(cs):
                        d0, nb = CHUNKS[c]
                        sl = slice(512 * k, 512 * (k + 1))
                        e0 = (d0 - 1) * N_NODES
                        nc.tensor.matmul(
                            pm4[:, sl], w4s,
                            _ap(nT9, d0, [[65, nb], [1, N_NODES]]),
                            start=True, stop=False,
                        )
                        nc.tensor.matmul(
                            pm4[:, sl], w4r, nT9[:, 0:512],
                            start=False, stop=False,
                        )
                        nc.tensor.matmul(
                            pm4[:, sl], w2bk, h2T[:, e0:e0 + 512],
                            start=False, stop=True,
                        )
                    h4T = h4p.tile([H, 1024], F32R, tag="h4T")
                    if p < 2:
                        nc.scalar.activation(
                            h4T, pm4, _AF.Relu, bias=biases["b4a"],
                        )
                    else:
                        nc.vector.tensor_scalar(
                            h4T, pm4, biases["b4a"], 0.0, _ALU.add, _ALU.max,
                        )
                    h4s.append(h4T)
                # output layer: one [16,512] matmul per chunk; evacuations
                # split between ACT and DVE
                for c in range(8):
                    h4T = h4s[c // 2]
                    hsl = slice(512 * (c % 2), 512 * (c % 2 + 1))
                    po = ppo.tile([N_OUT, 512], F32, tag="ppo")
                    nc.tensor.matmul(po, w4o, h4T[:, hsl], start=True, stop=True)
                    s0 = 512 * c if c < 7 else 3520
                    if c % 2 == 0:
                        nc.scalar.activation(
                            outP[:, s0:s0 + 512], po, _AF.Identity, bias=0.0
                        )
                    else:
                        nc.vector.tensor_copy(outP[:, s0:s0 + 512], po)
                nc.sync.dma_start(y_d[b], outP)

            for b in range(B_LOC):
                phase_a(b)
                if b >= 2:
                    phase_b(b - 2)
            phase_b(B_LOC - 2)
            phase_b(B_LOC - 1)

    nc.compile()
    return nc


_CACHE = {}


def _get_nc():
    if "nc" not in _CACHE:
        _CACHE["nc"] = build_kernel()
        _CACHE["perm"] = _edge_perm()
    return _CACHE["nc"], _CACHE["perm"]


def make_in_maps(inputs):
    w2b = np.asarray(inputs["w2b"], np.float32)
    w4a = np.asarray(inputs["w4a"], np.float32)
    b2b = np.asarray(inputs["b2b"], np.float32)
    w4a_k = w4a[2 * H:]
    def pad128(a):
        out = np.zeros((H, a.shape[1]), np.float32)
        out[:a.shape[0]] = a
        return out

    wblocks = [
        pad128(np.asarray(inputs["w1a"], np.float32)),
        inputs["w1b"], inputs["w2a"][:H], inputs["w2a"][H:], w2b,
        inputs["w3a"], inputs["w3b"], w4a[:H], w4a[H:2 * H], w2b @ w4a_k,
        pad128(np.pad((np.asarray(inputs["w4b"], np.float32) @ inputs["wout"]),
                      ((0, 0), (0, H - N_OUT)))),
    ]
    bcols = [
        inputs["b1a"], inputs["b1b"], inputs["b2a"],
        63.0 * b2b / (63.0 + 1e-6), inputs["b3a"], inputs["b3b"],
        np.asarray(inputs["b4a"], np.float32) + b2b @ w4a_k,
    ]
    wfix = np.concatenate(
        [np.ascontiguousarray(w, np.float32) for w in wblocks]
        + [np.asarray(v, np.float32)[:, None] for v in bcols], axis=1)
    x = np.ascontiguousarray(inputs["x"], np.float32)
    in_maps = []
    for c in range(N_CORES):
        xs = x[c * B_LOC:(c + 1) * B_LOC]
        xT = pad128(xs.reshape(B_LOC * N_NODES, N_IN).T)
        in_maps.append({"wpack": np.ascontiguousarray(
            np.concatenate([wfix, xT], axis=1))})
    return in_maps


def gather_out(results, perm, inputs):
    b4o = (np.asarray(inputs["b4b"], np.float32) @ inputs["wout"]
           + inputs["bout"]).astype(np.float32)  # [16]
    inv = np.empty_like(perm)
    inv[perm] = np.arange(N_EDGES)
    out = np.empty((BATCH, N_EDGES, N_OUT), np.float32)
    for cr in range(N_CORES):
        y = np.asarray(results[cr]["y"])  # [B_LOC, 16, 4032]
        full = y.transpose(0, 2, 1) + b4o  # [B_LOC, E'', 16]
        out[cr * B_LOC:(cr + 1) * B_LOC] = full[:, inv, :]
    return out


def kernel(**inputs):
    nc, perm = _get_nc()
    in_maps = make_in_maps(inputs)
    res = run_bass_kernel_spmd(nc, in_maps, core_ids=list(range(N_CORES)))
    return gather_out(res.results, perm, inputs)
